# revision 18
# baseline (speedup 1.0000x reference)
"""ComplEx forward (BN + fused GEMM + sigmoid) on 8 TRN2 NeuronCores.

Strategy (entity-parallel, per the sharding hint):
  - all_ent_emb [100000, 512] is sharded row-wise: 12500 entities per core,
    zero-padded to 12800 so each core runs 25 uniform chunks of 512.
  - e1/rel/gamma/beta are replicated; every core redundantly computes the two
    BatchNorms and combined = bn0(e1)*(re_rel+im_rel) + bn1(e1)*(re_rel-im_rel)
    in a cheap prologue (d-major layout so BN reduces along the free axis).
  - Prologue transposes the ent slab ONCE: the first n_res=12 chunks stay
    SBUF-resident for the whole kernel (zero steady-state load traffic);
    the rest land in a d-major scratch DRAM tensor and stream back as
    2KB-contiguous 1MB DMAs.  The PE does pure fp32r matmul on the
    critical path (no per-chunk transposes).
  - Sigmoid runs on the scalar engine over 4-PSUM-bank [128, 2048] tiles
    (50 ACTIVATEs per pass) writing fp16 into a per-chunk [128, 8, 512]
    tile that leaves as ONE 512KB DMA (36 DMAs/pass total) — per-DMA
    fixed cost, not bytes, was the measured limiter above the PE floor.

Layout notes:
  - The contraction dim (d=512) must sit on SBUF partitions for both matmul
    operands; entT provides that for ent, combT (built in the prologue) for
    the batch side.
  - Output orientation is [b-partitions, entity-free]; each [128, 8, 512]
    fp16 store covers all 8 batch tiles (1KB contiguous per DRAM row).
"""

import sys

if "/opt/trn_rl_repo" not in sys.path:
    sys.path.insert(0, "/opt/trn_rl_repo")

import numpy as np
from contextlib import ExitStack

import concourse.bass as bass
import concourse.tile as tile
from concourse import bacc, mybir
from concourse.bass_utils import run_bass_kernel_spmd
from concourse.masks import make_identity

P = 128
B = 1024  # batch
D = 512  # embedding dim
KT = D // P  # 4 k-tiles
BT = B // P  # 8 b-tiles
N_TOTAL = 100000
NCORES = 8
N_REAL = N_TOTAL // NCORES  # 12500 real entities per core
NCHUNK = 512
CHUNKS = 25
N_SLAB = NCHUNK * CHUNKS  # 12800 (padded)
GB_GROUP = 4  # bt groups per PSUM tile / ACT call
BN_EPS = 1e-5

F32 = mybir.dt.float32
F32R = mybir.dt.float32r
F16 = mybir.dt.float16
F8E3 = mybir.dt.float8e3

_CACHE = {}


def _build(repeat=1, mm_dtype=F32R, out_dtype=F16, ent_dtype=None, n_res=12,
           drop=()):
    """drop: subset of {"mm","act","outdma","indma"} — ablation timing builds.
    ent_dtype: dtype for the entT scratch/moving operand (default mm_dtype).
    n_res: number of 512-entity chunks kept SBUF-resident (no per-pass DMA)."""
    if ent_dtype is None:
        ent_dtype = mm_dtype
    nc = bacc.Bacc(None, target_bir_lowering=False)

    e1 = nc.dram_tensor("e1", [B, D], F32, kind="ExternalInput")
    rel = nc.dram_tensor("rel", [B, D], F32, kind="ExternalInput")
    ent = nc.dram_tensor("ent", [N_SLAB, D], F32, kind="ExternalInput")
    # gamma0, beta0, gamma1, beta1 stacked host-side
    gb = nc.dram_tensor("gb", [4, D], F32, kind="ExternalInput")
    entT_d = nc.dram_tensor("entTd", [D, N_SLAB], ent_dtype, kind="Internal")
    out = nc.dram_tensor("out", [B, N_SLAB], out_dtype, kind="ExternalOutput")

    out_pbn = out[:].rearrange("(g p) n -> p g n", p=P)
    entT_dv = entT_d[:].rearrange("(k p) n -> p k n", p=P)
    e1_v = e1[:].rearrange("(bt p) d -> p bt d", p=P)
    rel_v = rel[:].rearrange("(bt p) d -> p bt d", p=P)

    with tile.TileContext(nc) as tc:
        with ExitStack() as ctx:
            const = ctx.enter_context(tc.tile_pool(name="const", bufs=1))
            bnp = ctx.enter_context(tc.tile_pool(name="bnp", bufs=1))
            big = ctx.enter_context(tc.tile_pool(name="big", bufs=1))
            res_p = ctx.enter_context(tc.tile_pool(name="resident", bufs=1))
            entbm_p = ctx.enter_context(tc.tile_pool(name="entbm", bufs=2))
            enttp_p = ctx.enter_context(tc.tile_pool(name="enttp", bufs=2))
            entT_p = ctx.enter_context(tc.tile_pool(name="entT", bufs=2))
            outsb_p = ctx.enter_context(tc.tile_pool(name="outsb", bufs=2))
            # ALL of PSUM: 2 bufs x [128, 4 banks]
            pso = ctx.enter_context(tc.tile_pool(name="pso", bufs=2, space="PSUM"))

            ident = const.tile([P, P], F32)
            make_identity(nc, ident)
            eps_t = const.tile([P, 1], F32)
            nc.vector.memset(eps_t, BN_EPS)

            # gamma/beta transposed load straight from DRAM (tiny, once)
            gbT = const.tile([P, KT, 4], F32)
            gb_v = gb[:].rearrange("g (k p) -> p k g", p=P)
            for k in range(KT):
                nc.sync.dma_start(gbT[:, k, :], gb_v[:, k, :])

            combT = big.tile([P, KT, B], mm_dtype, tag="combT")

            # ------- prologue: BN + combined, one k-tile (128 dims) at a time -------
            for k in range(KT):
                e1_k = bnp.tile([P, BT, P], F32, tag="e1k")
                rel_k = bnp.tile([P, BT, P], F32, tag="relk")
                nc.sync.dma_start(e1_k[:], e1_v[:, :, k * P : (k + 1) * P])
                nc.sync.dma_start(rel_k[:], rel_v[:, :, k * P : (k + 1) * P])

                # transpose [128b, 128d] -> [128d, 128b]; slot bt of the PSUM
                # tile = [:, bt // 4, (bt % 4)*128 :], rel in the upper half
                ps_e = pso.tile([P, GB_GROUP, NCHUNK], F32, tag="pso")
                for bt in range(BT):
                    nc.tensor.transpose(
                        ps_e[:, bt // 4, (bt % 4) * P : (bt % 4 + 1) * P],
                        e1_k[:, bt, :],
                        ident[:],
                    )
                for bt in range(BT):
                    nc.tensor.transpose(
                        ps_e[:, 2 + bt // 4, (bt % 4) * P : (bt % 4 + 1) * P],
                        rel_k[:, bt, :],
                        ident[:],
                    )
                e1T_k = bnp.tile([P, B], F32, tag="e1Tk")  # -> re -> re*s
                relT_k = bnp.tile([P, B], F32, tag="relTk")  # -> s
                imT_k = bnp.tile([P, B], F32, tag="imTk")
                dT_k = bnp.tile([P, B], F32, tag="dTk")
                nc.vector.tensor_copy(
                    e1T_k[:].rearrange("p (h f) -> p h f", h=2), ps_e[:, 0:2, :]
                )
                nc.vector.tensor_copy(
                    relT_k[:].rearrange("p (h f) -> p h f", h=2), ps_e[:, 2:4, :]
                )

                # BN stats over the batch (free axis), 2 subgroups of 512
                stats = const.tile([P, 2, 6], F32, tag="bnstats")
                xk = e1T_k[:].rearrange("p (s f) -> p s f", s=2)
                nc.vector.bn_stats(stats[:, 0, :], xk[:, 0, :])
                nc.vector.bn_stats(stats[:, 1, :], xk[:, 1, :])
                mv = const.tile([P, 2], F32, tag="bnmv")
                nc.vector.bn_aggr(mv[:], stats[:])
                mean = mv[:, 0:1]
                var = mv[:, 1:2]
                rstd = const.tile([P, 1], F32, tag="rstd")
                nc.scalar.activation(
                    rstd[:], var, mybir.ActivationFunctionType.Sqrt, bias=eps_t[:]
                )
                nc.vector.reciprocal(rstd[:], rstd[:])

                # re = e1 * a0 + b0',  a0 = rstd*gamma0, b0' = beta0 - mean*a0
                a0 = const.tile([P, 1], F32, tag="a0")
                b0p = const.tile([P, 1], F32, tag="b0p")
                a1 = const.tile([P, 1], F32, tag="a1")
                b1p = const.tile([P, 1], F32, tag="b1p")
                nc.vector.tensor_mul(a0[:], rstd[:], gbT[:, k, 0:1])
                nc.vector.tensor_mul(b0p[:], mean, a0[:])
                nc.vector.tensor_tensor(
                    b0p[:], gbT[:, k, 1:2], b0p[:], mybir.AluOpType.subtract
                )
                nc.vector.tensor_mul(a1[:], rstd[:], gbT[:, k, 2:3])
                nc.vector.tensor_mul(b1p[:], mean, a1[:])
                nc.vector.tensor_tensor(
                    b1p[:], gbT[:, k, 3:4], b1p[:], mybir.AluOpType.subtract
                )

                # im first (needs raw e1T_k), then re in-place over e1T_k
                nc.vector.tensor_scalar(
                    imT_k[:],
                    e1T_k[:],
                    scalar1=a1[:],
                    scalar2=b1p[:],
                    op0=mybir.AluOpType.mult,
                    op1=mybir.AluOpType.add,
                )
                nc.vector.tensor_scalar(
                    e1T_k[:],
                    e1T_k[:],
                    scalar1=a0[:],
                    scalar2=b0p[:],
                    op0=mybir.AluOpType.mult,
                    op1=mybir.AluOpType.add,
                )
                # re_rel = im_rel = rel: s = re_rel + im_rel, d = re_rel - im_rel
                nc.vector.tensor_tensor(
                    dT_k[:], relT_k[:], relT_k[:], mybir.AluOpType.subtract
                )
                nc.vector.tensor_tensor(
                    relT_k[:], relT_k[:], relT_k[:], mybir.AluOpType.add
                )
                # combT = re*s + im*d  (final add writes the mm-dtype tile)
                nc.vector.tensor_mul(e1T_k[:], e1T_k[:], relT_k[:])
                nc.vector.tensor_mul(imT_k[:], imT_k[:], dT_k[:])
                nc.vector.tensor_add(combT[:, k, :], e1T_k[:], imT_k[:])

            # -------- prologue: transpose ent; first n_res chunks stay SBUF- --------
            # -------- resident, the rest go to d-major scratch DRAM         --------
            entT_res = [
                res_p.tile([P, KT, NCHUNK], ent_dtype, tag=f"res{i}", name=f"res{i}")
                for i in range(n_res)
            ]
            ent_pbn = ent[:].rearrange("(c nt p) d -> c p nt d", p=P, nt=NCHUNK // P)
            for ci in range(CHUNKS):
                ent_bm = entbm_p.tile([P, NCHUNK // P, D], F32, tag="ent_bm")
                nc.sync.dma_start(ent_bm[:], ent_pbn[ci])
                ps_t = pso.tile([P, GB_GROUP, NCHUNK], F32, tag="pso")
                for k in range(KT):
                    for nt in range(NCHUNK // P):
                        nc.tensor.transpose(
                            ps_t[:, k, nt * P : (nt + 1) * P],
                            ent_bm[:, nt, k * P : (k + 1) * P],
                            ident[:],
                        )
                if ci < n_res:
                    nc.vector.tensor_copy(entT_res[ci][:], ps_t[:])
                else:
                    entT_sb = enttp_p.tile([P, KT, NCHUNK], ent_dtype, tag="entT_sb")
                    nc.vector.tensor_copy(entT_sb[:], ps_t[:])
                    nc.sync.dma_start(
                        entT_dv[:, :, ci * NCHUNK : (ci + 1) * NCHUNK], entT_sb[:]
                    )

            # ---------------- main loop over entity chunks ----------------
            for ci in [c for _ in range(repeat) for c in range(CHUNKS)]:
                if ci < n_res:
                    entT = entT_res[ci]
                elif "indma" not in drop:
                    entT = entT_p.tile([P, KT, NCHUNK], ent_dtype, tag="entT")
                    nc.sync.dma_start(
                        entT[:], entT_dv[:, :, ci * NCHUNK : (ci + 1) * NCHUNK]
                    )
                else:
                    entT = combT  # ablation: read a resident tile instead

                if "act" not in drop:
                    out_sb = outsb_p.tile(
                        [P, BT, NCHUNK], out_dtype, tag="out_sb", name="out_sb"
                    )
                else:
                    out_sb = None
                for half in range(BT // GB_GROUP):
                    if "mm" not in drop:
                        ps_o = pso.tile([P, GB_GROUP, NCHUNK], F32, tag="pso")
                        for g in range(GB_GROUP):
                            bt = half * GB_GROUP + g
                            for k in range(KT):
                                nc.tensor.matmul(
                                    ps_o[:, g, :],
                                    combT[:, k, bt * P : (bt + 1) * P],
                                    entT[:, k, :NCHUNK],
                                    start=(k == 0),
                                    stop=(k == KT - 1),
                                )
                        act_src = ps_o
                    else:
                        act_src = None
                    if "act" not in drop:
                        dst = out_sb[:, half * GB_GROUP : (half + 1) * GB_GROUP, :]
                        if act_src is not None:
                            src = act_src[:]
                        else:  # ablation: resident SBUF tile
                            src = combT[:, :, :NCHUNK]
                        if out_dtype == F8E3:
                            # fp8 can't hold sigmoid near 1.0; store
                            # tanh(x/2) = 2*sigmoid(x)-1 instead (host undoes)
                            nc.scalar.activation(
                                dst,
                                src,
                                mybir.ActivationFunctionType.Tanh,
                                scale=0.5,
                            )
                        else:
                            nc.scalar.activation(
                                dst, src, mybir.ActivationFunctionType.Sigmoid
                            )
                # one store per chunk (fewer, larger DMAs); keep it on SP's
                # ring — issuing from nc.scalar steals ACT cycles (measured
                # +34us/pass sustained)
                if "act" not in drop and "outdma" not in drop:
                    nc.sync.dma_start(
                        out_pbn[:, :, ci * NCHUNK : (ci + 1) * NCHUNK], out_sb[:]
                    )

    nc.compile()
    return nc


def _get_nc(repeat=1, mm_dtype=F32R, out_dtype=F16, ent_dtype=None, n_res=12,
            drop=()):
    key = f"nc{repeat}_{mm_dtype}_{out_dtype}_{ent_dtype}_{n_res}_{sorted(drop)}"
    if key not in _CACHE:
        _CACHE[key] = _build(
            repeat,
            mm_dtype=mm_dtype,
            out_dtype=out_dtype,
            ent_dtype=ent_dtype,
            n_res=n_res,
            drop=drop,
        )
    return _CACHE[key]


def _prep_per_core(inputs):
    e1 = np.ascontiguousarray(np.asarray(inputs["e1_emb"], dtype=np.float32))
    rel = np.ascontiguousarray(np.asarray(inputs["rel_emb"], dtype=np.float32))
    ent = np.ascontiguousarray(np.asarray(inputs["all_ent_emb"], dtype=np.float32))
    gb = np.ascontiguousarray(
        np.stack(
            [
                np.asarray(inputs["gamma0"], dtype=np.float32),
                np.asarray(inputs["beta0"], dtype=np.float32),
                np.asarray(inputs["gamma1"], dtype=np.float32),
                np.asarray(inputs["beta1"], dtype=np.float32),
            ]
        )
    )
    per_core = []
    for c in range(NCORES):
        shard = np.zeros((N_SLAB, D), dtype=np.float32)
        shard[:N_REAL] = ent[c * N_REAL : (c + 1) * N_REAL]
        per_core.append({"e1": e1, "rel": rel, "ent": shard, "gb": gb})
    return per_core


def _run(inputs, trace=False, trace_kwargs=None, **build_kw):
    in_maps = _prep_per_core(inputs)
    nc = _get_nc(1, **build_kw)
    kwargs = {}
    if trace:
        kwargs["trace"] = True
        if trace_kwargs:
            kwargs.update(trace_kwargs)
    res = run_bass_kernel_spmd(nc, in_maps, core_ids=list(range(NCORES)), **kwargs)
    full = np.concatenate(
        [
            np.asarray(res.results[c]["out"][:, :N_REAL]).astype(np.float32)
            for c in range(NCORES)
        ],
        axis=1,
    )
    if build_kw.get("out_dtype", F16) == F8E3:
        full = 0.5 + 0.5 * full  # undo the tanh(x/2) encoding
    return full, res


def kernel(**inputs):
    full, _ = _run(inputs)
    return full


def _make_sharded(nc, n_cores=NCORES):
    """Replicate run_bass_via_pjrt's multi-core jit so we can time repeated
    executions with device-resident inputs (NTFF profiling is unavailable
    under this axon client)."""
    import jax
    from jax.sharding import Mesh, PartitionSpec
    from jax.experimental.shard_map import shard_map
    from concourse import bass2jax as b2j

    b2j.install_neuronx_cc_hook()

    partition_name = nc.partition_id_tensor.name if nc.partition_id_tensor else None
    in_names, out_names, out_avals = [], [], []
    for alloc in nc.m.functions[0].allocations:
        if not isinstance(alloc, mybir.MemoryLocationSet):
            continue
        name = alloc.memorylocations[0].name
        if alloc.kind == "ExternalInput":
            if name != partition_name:
                in_names.append(name)
        elif alloc.kind == "ExternalOutput":
            out_names.append(name)
            shape = tuple(alloc.tensor_shape)
            dtype = mybir.dt.np(alloc.dtype)
            out_avals.append(jax.core.ShapedArray(shape, dtype))
    n_params = len(in_names)
    n_outs = len(out_avals)
    all_in_names = list(in_names) + list(out_names)
    if partition_name is not None:
        all_in_names.append(partition_name)

    donate = tuple(range(n_params, n_params + n_outs))

    def _body(*args):
        operands = list(args)
        if partition_name is not None:
            operands.append(b2j.partition_id_tensor())
        outs = b2j._bass_exec_p.bind(
            *operands,
            out_avals=tuple(out_avals),
            in_names=tuple(all_in_names),
            out_names=tuple(out_names),
            lowering_input_output_aliases=(),
            sim_require_finite=True,
            sim_require_nnan=True,
            nc=nc,
        )
        return tuple(outs)

    devices = jax.devices()[:n_cores]
    mesh = Mesh(np.asarray(devices), ("core",))
    in_specs = (PartitionSpec("core"),) * (n_params + n_outs)
    out_specs = (PartitionSpec("core"),) * n_outs
    sharded = jax.jit(
        shard_map(
            _body, mesh=mesh, in_specs=in_specs, out_specs=out_specs, check_rep=False
        ),
        donate_argnums=donate,
        keep_unused=True,
    )
    return sharded, in_names, out_names, out_avals


class _TimedRunner:
    """Warm jit-callable for one nc with device-resident, pre-sharded inputs."""

    def __init__(self, nc, per_core):
        import jax
        from jax.sharding import Mesh, NamedSharding, PartitionSpec

        self.jax = jax
        sharded, in_names, out_names, out_avals = _make_sharded(nc)
        self.sharded = sharded
        self.out_avals = out_avals
        mesh = Mesh(np.asarray(jax.devices()[:NCORES]), ("core",))
        self.shd = NamedSharding(mesh, PartitionSpec("core"))
        concat_in = [
            np.concatenate([per_core[c][nm] for c in range(NCORES)], axis=0)
            for nm in in_names
        ]
        self.dev_in = [jax.device_put(a, self.shd) for a in concat_in]
        jax.block_until_ready(self.dev_in)
        self._zeros_np = [
            np.zeros((NCORES * av.shape[0], *av.shape[1:]), av.dtype)
            for av in out_avals
        ]

    def run(self):
        import time

        jax = self.jax
        zeros = [jax.device_put(z, self.shd) for z in self._zeros_np]
        jax.block_until_ready(zeros)
        t0 = time.perf_counter()
        outs = self.sharded(*self.dev_in, *zeros)
        jax.block_until_ready(outs)
        t1 = time.perf_counter()
        for o in outs:
            o.delete()
        return (t1 - t0) * 1e9


def benchmark(inputs, iters=8, repeat=9, **build_kw):
    """Estimate per-invocation HW time by comparing a kernel that runs the
    main loop once vs `repeat` times (dispatch overhead cancels).
    Returns (times_1, times_R, repeat)."""
    per_core = _prep_per_core(inputs)
    r1 = _TimedRunner(_get_nc(1, **build_kw), per_core)
    rR = _TimedRunner(_get_nc(repeat, **build_kw), per_core)
    for _ in range(3):
        r1.run()
        rR.run()
    t1s, tRs = [], []
    for _ in range(iters):
        t1s.append(r1.run())
        tRs.append(rR.run())
    return t1s, tRs, repeat


if __name__ == "__main__":
    rng = np.random.default_rng(0)
    ins = {
        "e1_emb": rng.standard_normal((B, D), dtype=np.float32),
        "rel_emb": rng.standard_normal((B, D), dtype=np.float32),
        "all_ent_emb": rng.standard_normal((N_TOTAL, D), dtype=np.float32),
        "gamma0": np.ones(D, np.float32),
        "beta0": np.zeros(D, np.float32),
        "gamma1": np.ones(D, np.float32),
        "beta1": np.zeros(D, np.float32),
    }
    out = kernel(**ins)
    print("out", out.shape, out.dtype, out.min(), out.max())


# revision 20
# speedup vs baseline: 2.3224x; 2.3224x over previous
"""ComplEx forward (BN + fused GEMM + sigmoid) on 8 TRN2 NeuronCores.

Strategy (entity-parallel, per the sharding hint):
  - all_ent_emb [100000, 512] is sharded row-wise: 12500 entities per core,
    zero-padded to 12544 = 24 chunks of 512 entities plus one of 256.
  - e1/rel/gamma/beta are replicated; every core redundantly computes the two
    BatchNorms and combined = bn0(e1)*(re_rel+im_rel) + bn1(e1)*(re_rel-im_rel)
    in a cheap prologue (d-major layout so BN reduces along the free axis).
  - Prologue transposes the ent slab ONCE: the first n_res=12 chunks stay
    SBUF-resident for the whole kernel (zero steady-state load traffic);
    the rest land in a d-major scratch DRAM tensor and stream back as
    2KB-contiguous 1MB DMAs.  The PE does pure fp32r matmul on the
    critical path (no per-chunk transposes).
  - Sigmoid runs on the scalar engine over 4-PSUM-bank [128, 2048] tiles
    (50 ACTIVATEs per pass) writing fp16 into a per-chunk [128, 8, 512]
    tile that leaves as ONE 512KB DMA (36 DMAs/pass total) — per-DMA
    fixed cost, not bytes, was the measured limiter above the PE floor.

Layout notes:
  - The contraction dim (d=512) must sit on SBUF partitions for both matmul
    operands; entT provides that for ent, combT (built in the prologue) for
    the batch side.
  - Output orientation is [b-partitions, entity-free]; each [128, 8, 512]
    fp16 store covers all 8 batch tiles (1KB contiguous per DRAM row).
"""

import sys

if "/opt/trn_rl_repo" not in sys.path:
    sys.path.insert(0, "/opt/trn_rl_repo")

import numpy as np
from contextlib import ExitStack

import concourse.bass as bass
import concourse.tile as tile
from concourse import bacc, mybir
from concourse.bass_utils import run_bass_kernel_spmd
from concourse.masks import make_identity

P = 128
B = 1024  # batch
D = 512  # embedding dim
KT = D // P  # 4 k-tiles
BT = B // P  # 8 b-tiles
N_TOTAL = 100000
NCORES = 8
N_REAL = N_TOTAL // NCORES  # 12500 real entities per core
NCHUNK = 512
CHUNKS = 25
W_LAST = 256  # last chunk is half-width: 24*512 + 256 = 12544 >= 12500
N_SLAB = (CHUNKS - 1) * NCHUNK + W_LAST


def _cw(ci):
    return W_LAST if ci == CHUNKS - 1 else NCHUNK
GB_GROUP = 4  # bt groups per PSUM tile / ACT call
BN_EPS = 1e-5

F32 = mybir.dt.float32
F32R = mybir.dt.float32r
F16 = mybir.dt.float16
F8E3 = mybir.dt.float8e3

_CACHE = {}


def _build(repeat=1, mm_dtype=F32R, out_dtype=F16, ent_dtype=None, n_res=12,
           drop=()):
    """drop: subset of {"mm","act","outdma","indma"} — ablation timing builds.
    ent_dtype: dtype for the entT scratch/moving operand (default mm_dtype).
    n_res: number of 512-entity chunks kept SBUF-resident (no per-pass DMA)."""
    if ent_dtype is None:
        ent_dtype = mm_dtype
    nc = bacc.Bacc(None, target_bir_lowering=False)

    e1 = nc.dram_tensor("e1", [B, D], F32, kind="ExternalInput")
    rel = nc.dram_tensor("rel", [B, D], F32, kind="ExternalInput")
    ent = nc.dram_tensor("ent", [N_SLAB, D], F32, kind="ExternalInput")
    # gamma0, beta0, gamma1, beta1 stacked host-side
    gb = nc.dram_tensor("gb", [4, D], F32, kind="ExternalInput")
    entT_d = nc.dram_tensor("entTd", [D, N_SLAB], ent_dtype, kind="Internal")
    out = nc.dram_tensor("out", [B, N_SLAB], out_dtype, kind="ExternalOutput")

    out_pbn = out[:].rearrange("(g p) n -> p g n", p=P)
    entT_dv = entT_d[:].rearrange("(k p) n -> p k n", p=P)
    e1_v = e1[:].rearrange("(bt p) d -> p bt d", p=P)
    rel_v = rel[:].rearrange("(bt p) d -> p bt d", p=P)

    with tile.TileContext(nc) as tc:
        with ExitStack() as ctx:
            const = ctx.enter_context(tc.tile_pool(name="const", bufs=1))
            bnp = ctx.enter_context(tc.tile_pool(name="bnp", bufs=1))
            big = ctx.enter_context(tc.tile_pool(name="big", bufs=1))
            res_p = ctx.enter_context(tc.tile_pool(name="resident", bufs=1))
            entbm_p = ctx.enter_context(tc.tile_pool(name="entbm", bufs=2))
            enttp_p = ctx.enter_context(tc.tile_pool(name="enttp", bufs=2))
            entT_p = ctx.enter_context(tc.tile_pool(name="entT", bufs=2))
            outsb_p = ctx.enter_context(tc.tile_pool(name="outsb", bufs=2))
            # ALL of PSUM: 2 bufs x [128, 4 banks]
            pso = ctx.enter_context(tc.tile_pool(name="pso", bufs=2, space="PSUM"))

            ident = const.tile([P, P], F32)
            make_identity(nc, ident)
            eps_t = const.tile([P, 1], F32)
            nc.vector.memset(eps_t, BN_EPS)

            # gamma/beta transposed load straight from DRAM (tiny, once)
            gbT = const.tile([P, KT, 4], F32)
            gb_v = gb[:].rearrange("g (k p) -> p k g", p=P)
            for k in range(KT):
                nc.sync.dma_start(gbT[:, k, :], gb_v[:, k, :])

            combT = big.tile([P, KT, B], mm_dtype, tag="combT")

            # ------- prologue: BN + combined, one k-tile (128 dims) at a time -------
            for k in range(KT):
                e1_k = bnp.tile([P, BT, P], F32, tag="e1k")
                rel_k = bnp.tile([P, BT, P], F32, tag="relk")
                nc.sync.dma_start(e1_k[:], e1_v[:, :, k * P : (k + 1) * P])
                nc.sync.dma_start(rel_k[:], rel_v[:, :, k * P : (k + 1) * P])

                # transpose [128b, 128d] -> [128d, 128b]; slot bt of the PSUM
                # tile = [:, bt // 4, (bt % 4)*128 :], rel in the upper half
                ps_e = pso.tile([P, GB_GROUP, NCHUNK], F32, tag="pso")
                for bt in range(BT):
                    nc.tensor.transpose(
                        ps_e[:, bt // 4, (bt % 4) * P : (bt % 4 + 1) * P],
                        e1_k[:, bt, :],
                        ident[:],
                    )
                for bt in range(BT):
                    nc.tensor.transpose(
                        ps_e[:, 2 + bt // 4, (bt % 4) * P : (bt % 4 + 1) * P],
                        rel_k[:, bt, :],
                        ident[:],
                    )
                e1T_k = bnp.tile([P, B], F32, tag="e1Tk")  # -> re -> re*s
                relT_k = bnp.tile([P, B], F32, tag="relTk")  # -> s
                imT_k = bnp.tile([P, B], F32, tag="imTk")
                dT_k = bnp.tile([P, B], F32, tag="dTk")
                nc.vector.tensor_copy(
                    e1T_k[:].rearrange("p (h f) -> p h f", h=2), ps_e[:, 0:2, :]
                )
                nc.vector.tensor_copy(
                    relT_k[:].rearrange("p (h f) -> p h f", h=2), ps_e[:, 2:4, :]
                )

                # BN stats over the batch (free axis), 2 subgroups of 512
                stats = const.tile([P, 2, 6], F32, tag="bnstats")
                xk = e1T_k[:].rearrange("p (s f) -> p s f", s=2)
                nc.vector.bn_stats(stats[:, 0, :], xk[:, 0, :])
                nc.vector.bn_stats(stats[:, 1, :], xk[:, 1, :])
                mv = const.tile([P, 2], F32, tag="bnmv")
                nc.vector.bn_aggr(mv[:], stats[:])
                mean = mv[:, 0:1]
                var = mv[:, 1:2]
                rstd = const.tile([P, 1], F32, tag="rstd")
                nc.scalar.activation(
                    rstd[:], var, mybir.ActivationFunctionType.Sqrt, bias=eps_t[:]
                )
                nc.vector.reciprocal(rstd[:], rstd[:])

                # re = e1 * a0 + b0',  a0 = rstd*gamma0, b0' = beta0 - mean*a0
                a0 = const.tile([P, 1], F32, tag="a0")
                b0p = const.tile([P, 1], F32, tag="b0p")
                a1 = const.tile([P, 1], F32, tag="a1")
                b1p = const.tile([P, 1], F32, tag="b1p")
                nc.vector.tensor_mul(a0[:], rstd[:], gbT[:, k, 0:1])
                nc.vector.tensor_mul(b0p[:], mean, a0[:])
                nc.vector.tensor_tensor(
                    b0p[:], gbT[:, k, 1:2], b0p[:], mybir.AluOpType.subtract
                )
                nc.vector.tensor_mul(a1[:], rstd[:], gbT[:, k, 2:3])
                nc.vector.tensor_mul(b1p[:], mean, a1[:])
                nc.vector.tensor_tensor(
                    b1p[:], gbT[:, k, 3:4], b1p[:], mybir.AluOpType.subtract
                )

                # im first (needs raw e1T_k), then re in-place over e1T_k
                nc.vector.tensor_scalar(
                    imT_k[:],
                    e1T_k[:],
                    scalar1=a1[:],
                    scalar2=b1p[:],
                    op0=mybir.AluOpType.mult,
                    op1=mybir.AluOpType.add,
                )
                nc.vector.tensor_scalar(
                    e1T_k[:],
                    e1T_k[:],
                    scalar1=a0[:],
                    scalar2=b0p[:],
                    op0=mybir.AluOpType.mult,
                    op1=mybir.AluOpType.add,
                )
                # re_rel = im_rel = rel: s = re_rel + im_rel, d = re_rel - im_rel
                nc.vector.tensor_tensor(
                    dT_k[:], relT_k[:], relT_k[:], mybir.AluOpType.subtract
                )
                nc.vector.tensor_tensor(
                    relT_k[:], relT_k[:], relT_k[:], mybir.AluOpType.add
                )
                # combT = re*s + im*d  (final add writes the mm-dtype tile)
                nc.vector.tensor_mul(e1T_k[:], e1T_k[:], relT_k[:])
                nc.vector.tensor_mul(imT_k[:], imT_k[:], dT_k[:])
                nc.vector.tensor_add(combT[:, k, :], e1T_k[:], imT_k[:])

            # -------- prologue: transpose ent; first n_res chunks stay SBUF- --------
            # -------- resident, the rest go to d-major scratch DRAM         --------
            entT_res = [
                res_p.tile([P, KT, NCHUNK], ent_dtype, tag=f"res{i}", name=f"res{i}")
                for i in range(n_res)
            ]
            ent_g = ent[:].rearrange("(g p) d -> g p d", p=P)
            for ci in range(CHUNKS):
                w = _cw(ci)
                nt_n = w // P
                ent_bm = entbm_p.tile([P, NCHUNK // P, D], F32, tag="ent_bm")
                for nt in range(nt_n):
                    nc.sync.dma_start(
                        ent_bm[:, nt, :], ent_g[ci * (NCHUNK // P) + nt]
                    )
                ps_t = pso.tile([P, GB_GROUP, NCHUNK], F32, tag="pso")
                for k in range(KT):
                    for nt in range(nt_n):
                        nc.tensor.transpose(
                            ps_t[:, k, nt * P : (nt + 1) * P],
                            ent_bm[:, nt, k * P : (k + 1) * P],
                            ident[:],
                        )
                if ci < n_res:
                    nc.vector.tensor_copy(entT_res[ci][:, :, :w], ps_t[:, :, :w])
                else:
                    entT_sb = enttp_p.tile([P, KT, NCHUNK], ent_dtype, tag="entT_sb")
                    nc.vector.tensor_copy(entT_sb[:, :, :w], ps_t[:, :, :w])
                    nc.sync.dma_start(
                        entT_dv[:, :, ci * NCHUNK : ci * NCHUNK + w],
                        entT_sb[:, :, :w],
                    )

            # ---------------- main loop over entity chunks ----------------
            for ci in [c for _ in range(repeat) for c in range(CHUNKS)]:
                w = _cw(ci)
                if ci < n_res:
                    entT = entT_res[ci]
                elif "indma" not in drop:
                    entT = entT_p.tile([P, KT, NCHUNK], ent_dtype, tag="entT")
                    nc.sync.dma_start(
                        entT[:, :, :w],
                        entT_dv[:, :, ci * NCHUNK : ci * NCHUNK + w],
                    )
                else:
                    entT = combT  # ablation: read a resident tile instead

                if "act" not in drop:
                    out_sb = outsb_p.tile(
                        [P, BT, NCHUNK], out_dtype, tag="out_sb", name="out_sb"
                    )
                else:
                    out_sb = None
                for half in range(BT // GB_GROUP):
                    if "mm" not in drop:
                        ps_o = pso.tile([P, GB_GROUP, NCHUNK], F32, tag="pso")
                        for g in range(GB_GROUP):
                            bt = half * GB_GROUP + g
                            for k in range(KT):
                                nc.tensor.matmul(
                                    ps_o[:, g, :w],
                                    combT[:, k, bt * P : (bt + 1) * P],
                                    entT[:, k, :w],
                                    start=(k == 0),
                                    stop=(k == KT - 1),
                                )
                        act_src = ps_o
                    else:
                        act_src = None
                    if "act" not in drop:
                        dst = out_sb[:, half * GB_GROUP : (half + 1) * GB_GROUP, :w]
                        if act_src is not None:
                            src = act_src[:, :, :w]
                        else:  # ablation: resident SBUF tile
                            src = combT[:, :, :w]
                        if out_dtype == F8E3:
                            # fp8 can't hold sigmoid near 1.0; store
                            # tanh(x/2) = 2*sigmoid(x)-1 instead (host undoes)
                            nc.scalar.activation(
                                dst,
                                src,
                                mybir.ActivationFunctionType.Tanh,
                                scale=0.5,
                            )
                        else:
                            nc.scalar.activation(
                                dst, src, mybir.ActivationFunctionType.Sigmoid
                            )
                # one store per chunk (fewer, larger DMAs); keep it on SP's
                # ring — issuing from nc.scalar steals ACT cycles (measured
                # +34us/pass sustained)
                if "act" not in drop and "outdma" not in drop:
                    nc.sync.dma_start(
                        out_pbn[:, :, ci * NCHUNK : ci * NCHUNK + w],
                        out_sb[:, :, :w],
                    )

    nc.compile()
    return nc


def _get_nc(repeat=1, mm_dtype=F32R, out_dtype=F16, ent_dtype=None, n_res=12,
            drop=()):
    key = f"nc{repeat}_{mm_dtype}_{out_dtype}_{ent_dtype}_{n_res}_{sorted(drop)}"
    if key not in _CACHE:
        _CACHE[key] = _build(
            repeat,
            mm_dtype=mm_dtype,
            out_dtype=out_dtype,
            ent_dtype=ent_dtype,
            n_res=n_res,
            drop=drop,
        )
    return _CACHE[key]


def _prep_per_core(inputs):
    e1 = np.ascontiguousarray(np.asarray(inputs["e1_emb"], dtype=np.float32))
    rel = np.ascontiguousarray(np.asarray(inputs["rel_emb"], dtype=np.float32))
    ent = np.ascontiguousarray(np.asarray(inputs["all_ent_emb"], dtype=np.float32))
    gb = np.ascontiguousarray(
        np.stack(
            [
                np.asarray(inputs["gamma0"], dtype=np.float32),
                np.asarray(inputs["beta0"], dtype=np.float32),
                np.asarray(inputs["gamma1"], dtype=np.float32),
                np.asarray(inputs["beta1"], dtype=np.float32),
            ]
        )
    )
    per_core = []
    for c in range(NCORES):
        shard = np.zeros((N_SLAB, D), dtype=np.float32)
        shard[:N_REAL] = ent[c * N_REAL : (c + 1) * N_REAL]
        per_core.append({"e1": e1, "rel": rel, "ent": shard, "gb": gb})
    return per_core


def _run(inputs, trace=False, trace_kwargs=None, **build_kw):
    in_maps = _prep_per_core(inputs)
    nc = _get_nc(1, **build_kw)
    kwargs = {}
    if trace:
        kwargs["trace"] = True
        if trace_kwargs:
            kwargs.update(trace_kwargs)
    res = run_bass_kernel_spmd(nc, in_maps, core_ids=list(range(NCORES)), **kwargs)
    full = np.concatenate(
        [
            np.asarray(res.results[c]["out"][:, :N_REAL]).astype(np.float32)
            for c in range(NCORES)
        ],
        axis=1,
    )
    if build_kw.get("out_dtype", F16) == F8E3:
        full = 0.5 + 0.5 * full  # undo the tanh(x/2) encoding
    return full, res


def kernel(**inputs):
    full, _ = _run(inputs)
    return full


def _make_sharded(nc, n_cores=NCORES):
    """Replicate run_bass_via_pjrt's multi-core jit so we can time repeated
    executions with device-resident inputs (NTFF profiling is unavailable
    under this axon client)."""
    import jax
    from jax.sharding import Mesh, PartitionSpec
    from jax.experimental.shard_map import shard_map
    from concourse import bass2jax as b2j

    b2j.install_neuronx_cc_hook()

    partition_name = nc.partition_id_tensor.name if nc.partition_id_tensor else None
    in_names, out_names, out_avals = [], [], []
    for alloc in nc.m.functions[0].allocations:
        if not isinstance(alloc, mybir.MemoryLocationSet):
            continue
        name = alloc.memorylocations[0].name
        if alloc.kind == "ExternalInput":
            if name != partition_name:
                in_names.append(name)
        elif alloc.kind == "ExternalOutput":
            out_names.append(name)
            shape = tuple(alloc.tensor_shape)
            dtype = mybir.dt.np(alloc.dtype)
            out_avals.append(jax.core.ShapedArray(shape, dtype))
    n_params = len(in_names)
    n_outs = len(out_avals)
    all_in_names = list(in_names) + list(out_names)
    if partition_name is not None:
        all_in_names.append(partition_name)

    donate = tuple(range(n_params, n_params + n_outs))

    def _body(*args):
        operands = list(args)
        if partition_name is not None:
            operands.append(b2j.partition_id_tensor())
        outs = b2j._bass_exec_p.bind(
            *operands,
            out_avals=tuple(out_avals),
            in_names=tuple(all_in_names),
            out_names=tuple(out_names),
            lowering_input_output_aliases=(),
            sim_require_finite=True,
            sim_require_nnan=True,
            nc=nc,
        )
        return tuple(outs)

    devices = jax.devices()[:n_cores]
    mesh = Mesh(np.asarray(devices), ("core",))
    in_specs = (PartitionSpec("core"),) * (n_params + n_outs)
    out_specs = (PartitionSpec("core"),) * n_outs
    sharded = jax.jit(
        shard_map(
            _body, mesh=mesh, in_specs=in_specs, out_specs=out_specs, check_rep=False
        ),
        donate_argnums=donate,
        keep_unused=True,
    )
    return sharded, in_names, out_names, out_avals


class _TimedRunner:
    """Warm jit-callable for one nc with device-resident, pre-sharded inputs."""

    def __init__(self, nc, per_core):
        import jax
        from jax.sharding import Mesh, NamedSharding, PartitionSpec

        self.jax = jax
        sharded, in_names, out_names, out_avals = _make_sharded(nc)
        self.sharded = sharded
        self.out_avals = out_avals
        mesh = Mesh(np.asarray(jax.devices()[:NCORES]), ("core",))
        self.shd = NamedSharding(mesh, PartitionSpec("core"))
        concat_in = [
            np.concatenate([per_core[c][nm] for c in range(NCORES)], axis=0)
            for nm in in_names
        ]
        self.dev_in = [jax.device_put(a, self.shd) for a in concat_in]
        jax.block_until_ready(self.dev_in)
        self._zeros_np = [
            np.zeros((NCORES * av.shape[0], *av.shape[1:]), av.dtype)
            for av in out_avals
        ]

    def run(self):
        import time

        jax = self.jax
        zeros = [jax.device_put(z, self.shd) for z in self._zeros_np]
        jax.block_until_ready(zeros)
        t0 = time.perf_counter()
        outs = self.sharded(*self.dev_in, *zeros)
        jax.block_until_ready(outs)
        t1 = time.perf_counter()
        for o in outs:
            o.delete()
        return (t1 - t0) * 1e9


def benchmark(inputs, iters=8, repeat=9, **build_kw):
    """Estimate per-invocation HW time by comparing a kernel that runs the
    main loop once vs `repeat` times (dispatch overhead cancels).
    Returns (times_1, times_R, repeat)."""
    per_core = _prep_per_core(inputs)
    r1 = _TimedRunner(_get_nc(1, **build_kw), per_core)
    rR = _TimedRunner(_get_nc(repeat, **build_kw), per_core)
    for _ in range(3):
        r1.run()
        rR.run()
    t1s, tRs = [], []
    for _ in range(iters):
        t1s.append(r1.run())
        tRs.append(rR.run())
    return t1s, tRs, repeat


if __name__ == "__main__":
    rng = np.random.default_rng(0)
    ins = {
        "e1_emb": rng.standard_normal((B, D), dtype=np.float32),
        "rel_emb": rng.standard_normal((B, D), dtype=np.float32),
        "all_ent_emb": rng.standard_normal((N_TOTAL, D), dtype=np.float32),
        "gamma0": np.ones(D, np.float32),
        "beta0": np.zeros(D, np.float32),
        "gamma1": np.ones(D, np.float32),
        "beta1": np.zeros(D, np.float32),
    }
    out = kernel(**ins)
    print("out", out.shape, out.dtype, out.min(), out.max())
